# revision 31
# baseline (speedup 1.0000x reference)
"""Bidirectional 4-layer SRU encoder on 8 TRN2 NeuronCores.

Strategy: data-parallel over batch (B=16 -> 2 per core). Per core and layer:
  u = x @ W computed on the tensor engine in fp32r (1 cycle/row, ~1.5e-4 rel
  err), with hidden channels on PSUM partitions and tokens (b, t) on the free
  dim. The SRU cell recurrence c[t] = f[t]*c[t-1] + (1-f[t])*xt[t] maps onto
  the DVE tensor_tensor_scan instruction along the free dim; the backward
  direction runs the same scan through reversed (negative-stride) APs.
  Activations (sigmoid/tanh) on the scalar engine, remaining elementwise on
  DVE/GpSimd.

Pipelining: per (direction, unit-group) pair the matmul order is u_f, u_xt,
  u_r so each PSUM buffer's consumers finish before the tensor engine needs
  the slot again, and the post-scan elementwise tail (tanh/sub/mul/add) of
  pair p is emitted during pair p+1 so it never head-of-line blocks the
  scalar engine's sigmoids.

Layouts (per core):
  x buffer  : [128 part, KT=8, T=2048] fp32r, channel c = k*128+p, free = b*L+t
  W         : [l, pair, g, p, k*128] fp32r; pair=(dir*4+j) selects 128 output
              units, g in {xt,f,r}; one contiguous 512KB chunk per quantity
  biases    : [128, (l*8+pair)*2 + {f,r}] fp32
Outputs: y [KT,128,T] fp32r bits (== fp32), hid [NL, pair, 128, BL] fp32.
Embedding gather happens host-side (pure memory reshuffle).
"""
import sys

sys.path.insert(0, "/opt/trn_rl_repo")

import numpy as np

L = 1024          # sequence length
B = 16            # global batch
NCORES = 8
BL = B // NCORES  # batch per core = 2
NOUT = 512        # hidden per direction
D = 2 * NOUT      # input dim = 1024
NL = 4            # layers
KT = D // 128     # contraction k-tiles = 8
NPAIR = NOUT // 128  # 128-unit groups per direction = 4
T = BL * L        # tokens per core = 2048
MMN = 512         # matmul moving free size
NCH = (T + MMN - 1) // MMN  # free chunks = 4

_cached = None


def _rev(ap_mod, ap):
    """Reversed view along the (single) free dim of a 2D [P, F] AP."""
    (pstep, pcount), (fstep, fcount) = ap.ap
    return ap_mod.AP(
        tensor=ap.tensor,
        offset=ap.offset + fstep * (fcount - 1),
        ap=[[pstep, pcount], [-fstep, fcount]],
    )


def _build():
    import concourse.bass as bass
    import concourse.bacc as bacc
    import concourse.mybir as mybir
    from concourse.tile import TileContext

    F32 = mybir.dt.float32
    F32R = mybir.dt.float32r
    AF = mybir.ActivationFunctionType
    OP = mybir.AluOpType

    nc = bacc.Bacc()
    x0_d = nc.declare_dram_parameter("x0", [KT, NCH, 128, MMN], F32R,
                                     isOutput=False)
    w_d = nc.declare_dram_parameter("w", [NL, 2 * NPAIR, 3, 128, KT * 128], F32R,
                                    isOutput=False)
    b_d = nc.declare_dram_parameter("bias", [128, NL * 2 * NPAIR * 2], F32,
                                    isOutput=False)
    y_d = nc.declare_dram_parameter("y", [KT, NCH, 128, MMN], F32R,
                                    isOutput=True)
    h_d = nc.declare_dram_parameter("hid", [NL, 2 * NPAIR, 128, BL], F32,
                                    isOutput=True)

    with TileContext(nc) as tc:
        with (
            tc.tile_pool(name="persist", bufs=1) as persist,
            tc.tile_pool(name="wp", bufs=3) as wp,
            tc.tile_pool(name="pp", bufs=2, space="PSUM") as pp,
            tc.tile_pool(name="ew", bufs=2) as ew,
        ):
            # per-(k, chunk) x buffers: Tile tracks SBUF deps at tile
            # granularity, so separate small tiles let each matmul start as
            # soon as the specific 256KB block it reads is resident
            # (prologue ramp + layer boundaries)
            xA = [[persist.tile([128, MMN], F32R, tag=f"xA{k}_{c}",
                                name=f"xA{k}_{c}") for c in range(NCH)]
                  for k in range(KT)]
            xB = [[persist.tile([128, MMN], F32R, tag=f"xB{k}_{c}",
                                name=f"xB{k}_{c}") for c in range(NCH)]
                  for k in range(KT)]
            barr = persist.tile([128, NL * 2 * NPAIR * 2], F32, tag="barr")
            barrn = persist.tile([128, NL * 2 * NPAIR * 2], F32, tag="barrn")

            # chunk-0 blocks first so the first matmul chains start after
            # ~2MB; issue spread across the three DMA-capable engine queues;
            # pair 0's weights slot in right after chunk 0 so the pipeline
            # ramps at DMA speed instead of waiting for the whole 8.4MB
            dma_eng = [nc.sync, nc.scalar, nc.gpsimd]
            for k in range(KT):
                dma_eng[k % 3].dma_start(out=xA[k][0][:], in_=x0_d[k, 0])
            nc.scalar.dma_start(out=barr[:], in_=b_d[:])
            pre_w = {}
            for g in (1, 0, 2):
                wt = wp.tile([128, KT * 128], F32R, tag="w", name=f"w0_{g}")
                nc.sync.dma_start(out=wt[:], in_=w_d[0, 0, g])
                pre_w[g] = wt
            for c in range(1, NCH):
                for k in range(KT):
                    dma_eng[k % 3].dma_start(out=xA[k][c][:], in_=x0_d[k, c])
            nc.vector.tensor_scalar_mul(barrn[:], barr[:], -1.0)

            # one-pair-delayed stage: scans + elementwise tail of pair p run
            # during pair p+1 so the only PSUM-gating consumers are the
            # shallow sigmoid/bscan reads right after each matmul block
            pend = [None]

            def tail(st):
                f_t, b_t, r_t, d, lq, pairq = st
                c_t = ew.tile([128, T], F32, tag="c")
                x_src, y_dst = (xA, xB) if lq % 2 == 0 else (xB, xA)
                # per b-segment: scan then elementwise tail, so the next
                # layer's first matmul chunks unblock as early as possible
                for b in range(BL):
                    seg = slice(b * L, (b + 1) * L)
                    o_ap, f_ap, b_ap = c_t[:, seg], f_t[:, seg], b_t[:, seg]
                    if d == 1:
                        o_ap, f_ap, b_ap = (_rev(bass, o_ap), _rev(bass, f_ap),
                                            _rev(bass, b_ap))
                    # c[t] = f[t]*c[t-1] + b[t]  (reversed for d=1)
                    nc.vector.tensor_tensor_scan(
                        o_ap, f_ap, b_ap, 0.0, OP.mult, OP.add)
                # final cell state per segment -> hidden output
                off = (L - 1) if d == 0 else 0
                hsrc = c_t[:].rearrange("p (b t) -> p b t", t=L)
                nc.sync.dma_start(out=h_d[lq, pairq],
                                  in_=hsrc[:, :, off:off + 1].squeeze(-1))
                # h = r*tanh(c) + (1-r)*xp = r*(tanh(c)-xp) + xp
                # tanh lands in the dead b_t buffer: an in-place write to c_t
                # would WAR-block the scalar engine behind the hid DMA above
                for b in range(BL):
                    seg = slice(b * L, (b + 1) * L)
                    nc.scalar.activation(b_t[:, seg], c_t[:, seg], AF.Tanh)
                    for ci in (2 * b, 2 * b + 1):
                        cs = slice(ci * MMN, (ci + 1) * MMN)
                        xp = x_src[pairq][ci][:].bitcast(F32)
                        nc.gpsimd.tensor_sub(b_t[:, cs], b_t[:, cs], xp)
                    nc.vector.tensor_mul(b_t[:, seg], r_t[:, seg], b_t[:, seg])
                    for ci in (2 * b, 2 * b + 1):
                        cs = slice(ci * MMN, (ci + 1) * MMN)
                        xp = x_src[pairq][ci][:].bitcast(F32)
                        nc.vector.tensor_add(y_dst[pairq][ci][:],
                                             b_t[:, cs], xp)
                        if lq == NL - 1:
                            # stream each finished block out immediately
                            nc.sync.dma_start(out=y_d[pairq, ci],
                                              in_=y_dst[pairq][ci][:])

            for l in range(NL):
                x_cur, y_cur = (xA, xB) if l % 2 == 0 else (xB, xA)
                for d in range(2):
                    for j in range(NPAIR):
                        pair = d * NPAIR + j

                        def mm(ps, g):
                            if l == 0 and pair == 0:
                                wt = pre_w.pop(g)
                            else:
                                wt = wp.tile([128, KT * 128], F32R, tag="w")
                                nc.sync.dma_start(out=wt[:], in_=w_d[l, pair, g])
                            for c in range(NCH):
                                cs = slice(c * MMN, (c + 1) * MMN)
                                for k in range(KT):
                                    nc.tensor.matmul(
                                        ps[:, cs],
                                        wt[:, k * 128:(k + 1) * 128],
                                        x_cur[k][c][:],
                                        start=(k == 0), stop=(k == KT - 1))

                        ps_f = pp.tile([128, T], F32, tag="ps")
                        mm(ps_f, 1)
                        ps_xt = pp.tile([128, T], F32, tag="ps")
                        mm(ps_xt, 0)

                        f_t = ew.tile([128, T], F32, tag="f")
                        b_t = ew.tile([128, T], F32, tag="b")
                        r_t = ew.tile([128, T], F32, tag="r")

                        colf = (l * 2 * NPAIR + pair) * 2
                        colr = colf + 1
                        # f = sigmoid(u_f + bf); b = (1-f)*xt = sigmoid(-u_f-bf)*xt
                        nc.scalar.activation(f_t[:], ps_f[:], AF.Sigmoid,
                                             bias=barr[:, colf:colf + 1])
                        nc.scalar.activation(b_t[:], ps_f[:], AF.Sigmoid,
                                             bias=barrn[:, colf:colf + 1],
                                             scale=-1.0)
                        nc.vector.tensor_mul(b_t[:], b_t[:], ps_xt[:])

                        ps_r = pp.tile([128, T], F32, tag="ps")
                        mm(ps_r, 2)

                        if pend[0] is not None:
                            tail(pend[0])

                        nc.scalar.activation(r_t[:], ps_r[:], AF.Sigmoid,
                                             bias=barr[:, colr:colr + 1])

                        pend[0] = (f_t, b_t, r_t, d, l, pair)
                # flush before the next layer reads this layer's outputs
                tail(pend[0])
                pend[0] = None

    nc.finalize()
    return nc


def _get_nc():
    global _cached
    if _cached is None:
        _cached = _build()
    return _cached


def _prep_inputs(rnn_input, emb, Ws, bs):
    """Host-side input arrangement -> per-core in_maps."""
    rnn_input = np.asarray(rnn_input)
    emb = np.asarray(emb, dtype=np.float32)
    Ws = np.asarray(Ws, dtype=np.float32)
    bs = np.asarray(bs, dtype=np.float32)

    x_full = emb[rnn_input]  # (L, B, D)

    # W: (NL, D, 6n) -> (l, pair, g, p, k*128) with col = d*3n + g*n + j*128 + c
    w_arr = np.empty((NL, 2 * NPAIR, 3, 128, KT * 128), np.float32)
    wv = Ws.reshape(NL, KT, 128, 6 * NOUT)  # (l, k, p, col)
    for d in range(2):
        for j in range(NPAIR):
            pair = d * NPAIR + j
            for g in range(3):
                base = d * 3 * NOUT + g * NOUT + j * 128
                blk = wv[:, :, :, base:base + 128]  # (NL, KT, 128p, 128c)
                # dest free index = k*128 + c, partition = p
                w_arr[:, pair, g] = blk.transpose(0, 2, 1, 3).reshape(
                    NL, 128, KT * 128)

    # biases: bs (NL, 4n): bf = [:2n], br = [2n:]
    b_arr = np.empty((128, NL * 2 * NPAIR * 2), np.float32)
    for l in range(NL):
        for d in range(2):
            for j in range(NPAIR):
                pair = d * NPAIR + j
                col = (l * 2 * NPAIR + pair) * 2
                b_arr[:, col] = bs[l, d * NOUT + j * 128:d * NOUT + (j + 1) * 128]
                b_arr[:, col + 1] = bs[l, 2 * NOUT + d * NOUT + j * 128:
                                       2 * NOUT + d * NOUT + (j + 1) * 128]

    in_maps = []
    for i in range(NCORES):
        xc = x_full[:, i * BL:(i + 1) * BL, :]       # (L, BL, D)
        xc = xc.transpose(2, 1, 0).reshape(KT, 128, NCH, MMN)
        xc = xc.transpose(0, 2, 1, 3)                # (k, c, p, t)
        in_maps.append({"x0": np.ascontiguousarray(xc), "w": w_arr, "bias": b_arr})
    return in_maps


def _assemble(results):
    xs, hs = [], []
    for res in results:
        y = res["y"].astype(np.float32)      # (KT, NCH, 128, MMN)
        y = y.transpose(0, 2, 1, 3).reshape(KT, 128, T)
        h = res["hid"].astype(np.float32)    # (NL, pair, 128, BL)
        xs.append(y.reshape(KT, 128, BL, L).transpose(3, 2, 0, 1).reshape(L, BL, D))
        hs.append(h.transpose(0, 3, 1, 2).reshape(NL, BL, D))
    x_out = np.concatenate(xs, axis=1)   # (L, B, D)
    hid_out = np.concatenate(hs, axis=1)  # (NL, B, D)
    return x_out, hid_out


def kernel(rnn_input, input_lengths, emb, Ws, bs):
    from concourse.bass_utils import run_bass_kernel_spmd

    in_maps = _prep_inputs(rnn_input, emb, Ws, bs)
    nc = _get_nc()
    out = run_bass_kernel_spmd(nc, in_maps, list(range(NCORES)))
    return _assemble(out.results)


# revision 33
# speedup vs baseline: 1.0736x; 1.0736x over previous
"""Bidirectional 4-layer SRU encoder on 8 TRN2 NeuronCores.

Strategy: data-parallel over batch (B=16 -> 2 per core). Per core and layer:
  u = x @ W computed on the tensor engine in fp32r (1 cycle/row, ~1.5e-4 rel
  err), with hidden channels on PSUM partitions and tokens (b, t) on the free
  dim. The SRU cell recurrence c[t] = f[t]*c[t-1] + (1-f[t])*xt[t] maps onto
  the DVE tensor_tensor_scan instruction along the free dim; the backward
  direction runs the same scan through reversed (negative-stride) APs.
  Activations (sigmoid/tanh) on the scalar engine, remaining elementwise on
  DVE/GpSimd.

Pipelining: per (direction, unit-group) pair the matmul order is u_f, u_xt,
  u_r so each PSUM buffer's consumers finish before the tensor engine needs
  the slot again, and the post-scan elementwise tail (tanh/sub/mul/add) of
  pair p is emitted during pair p+1 so it never head-of-line blocks the
  scalar engine's sigmoids.

Layouts (per core):
  x buffer  : [128 part, KT=8, T=2048] fp32r, channel c = k*128+p, free = b*L+t
  W         : [l, pair, g, p, k*128] fp32r; pair=(dir*4+j) selects 128 output
              units, g in {xt,f,r}; one contiguous 512KB chunk per quantity
  biases    : [128, (l*8+pair)*2 + {f,r}] fp32
Outputs: y [KT,128,T] fp32r bits (== fp32), hid [NL, pair, 128, BL] fp32.
Embedding gather happens host-side (pure memory reshuffle).
"""
import sys

sys.path.insert(0, "/opt/trn_rl_repo")

import numpy as np

L = 1024          # sequence length
B = 16            # global batch
NCORES = 8
BL = B // NCORES  # batch per core = 2
NOUT = 512        # hidden per direction
D = 2 * NOUT      # input dim = 1024
NL = 4            # layers
KT = D // 128     # contraction k-tiles = 8
NPAIR = NOUT // 128  # 128-unit groups per direction = 4
T = BL * L        # tokens per core = 2048
MMN = 512         # matmul moving free size
NCH = (T + MMN - 1) // MMN  # free chunks = 4

_cached = None


def _rev(ap_mod, ap):
    """Reversed view along the (single) free dim of a 2D [P, F] AP."""
    (pstep, pcount), (fstep, fcount) = ap.ap
    return ap_mod.AP(
        tensor=ap.tensor,
        offset=ap.offset + fstep * (fcount - 1),
        ap=[[pstep, pcount], [-fstep, fcount]],
    )


def _build():
    import concourse.bass as bass
    import concourse.bacc as bacc
    import concourse.mybir as mybir
    from concourse.tile import TileContext

    F32 = mybir.dt.float32
    F32R = mybir.dt.float32r
    AF = mybir.ActivationFunctionType
    OP = mybir.AluOpType

    nc = bacc.Bacc()
    x0_d = nc.declare_dram_parameter("x0", [KT, NCH, 128, MMN], F32R,
                                     isOutput=False)
    w_d = nc.declare_dram_parameter("w", [NL, 2 * NPAIR, 3, 128, KT * 128], F32R,
                                    isOutput=False)
    b_d = nc.declare_dram_parameter("bias", [128, NL * 2 * NPAIR * 2], F32,
                                    isOutput=False)
    y_d = nc.declare_dram_parameter("y", [KT, NCH, 128, MMN], F32R,
                                    isOutput=True)
    h_d = nc.declare_dram_parameter("hid", [NL, 2 * NPAIR, 128, BL], F32,
                                    isOutput=True)

    with TileContext(nc) as tc:
        with (
            tc.tile_pool(name="persist", bufs=1) as persist,
            tc.tile_pool(name="wp", bufs=3) as wp,
            tc.tile_pool(name="pp", bufs=2, space="PSUM") as pp,
            tc.tile_pool(name="ew", bufs=2) as ew,
        ):
            # per-(k, chunk) x buffers: Tile tracks SBUF deps at tile
            # granularity, so separate small tiles let each matmul start as
            # soon as the specific 256KB block it reads is resident
            # (prologue ramp + layer boundaries)
            xA = [[persist.tile([128, MMN], F32R, tag=f"xA{k}_{c}",
                                name=f"xA{k}_{c}") for c in range(NCH)]
                  for k in range(KT)]
            xB = [[persist.tile([128, MMN], F32R, tag=f"xB{k}_{c}",
                                name=f"xB{k}_{c}") for c in range(NCH)]
                  for k in range(KT)]
            barr = persist.tile([128, NL * 2 * NPAIR * 2], F32, tag="barr")
            barrn = persist.tile([128, NL * 2 * NPAIR * 2], F32, tag="barrn")

            # chunk-0 blocks first so the first matmul chains start after
            # ~2MB; issue spread across the three DMA-capable engine queues
            dma_eng = [nc.sync, nc.scalar, nc.gpsimd]
            for c in range(NCH):
                for k in range(KT):
                    dma_eng[k % 3].dma_start(out=xA[k][c][:], in_=x0_d[k, c])
            nc.scalar.dma_start(out=barr[:], in_=b_d[:])
            nc.vector.tensor_scalar_mul(barrn[:], barr[:], -1.0)

            # one-pair-delayed stage: scans + elementwise tail of pair p run
            # during pair p+1 so the only PSUM-gating consumers are the
            # shallow sigmoid/bscan reads right after each matmul block
            pend = [None]

            def tail(st):
                f_t, b_t, r_t, d, lq, pairq = st
                c_t = ew.tile([128, T], F32, tag="c")
                x_src, y_dst = (xA, xB) if lq % 2 == 0 else (xB, xA)
                # per b-segment: scan then elementwise tail, so the next
                # layer's first matmul chunks unblock as early as possible
                for b in range(BL):
                    seg = slice(b * L, (b + 1) * L)
                    o_ap, f_ap, b_ap = c_t[:, seg], f_t[:, seg], b_t[:, seg]
                    if d == 1:
                        o_ap, f_ap, b_ap = (_rev(bass, o_ap), _rev(bass, f_ap),
                                            _rev(bass, b_ap))
                    # c[t] = f[t]*c[t-1] + b[t]  (reversed for d=1)
                    nc.vector.tensor_tensor_scan(
                        o_ap, f_ap, b_ap, 0.0, OP.mult, OP.add)
                # final cell state per segment -> hidden output
                off = (L - 1) if d == 0 else 0
                hsrc = c_t[:].rearrange("p (b t) -> p b t", t=L)
                nc.sync.dma_start(out=h_d[lq, pairq],
                                  in_=hsrc[:, :, off:off + 1].squeeze(-1))
                # h = r*tanh(c) + (1-r)*xp = r*(tanh(c)-xp) + xp
                # tanh lands in the dead b_t buffer: an in-place write to c_t
                # would WAR-block the scalar engine behind the hid DMA above
                for b in range(BL):
                    seg = slice(b * L, (b + 1) * L)
                    nc.scalar.activation(b_t[:, seg], c_t[:, seg], AF.Tanh)
                    for ci in (2 * b, 2 * b + 1):
                        cs = slice(ci * MMN, (ci + 1) * MMN)
                        xp = x_src[pairq][ci][:].bitcast(F32)
                        nc.gpsimd.tensor_sub(b_t[:, cs], b_t[:, cs], xp)
                    nc.vector.tensor_mul(b_t[:, seg], r_t[:, seg], b_t[:, seg])
                    for ci in (2 * b, 2 * b + 1):
                        cs = slice(ci * MMN, (ci + 1) * MMN)
                        xp = x_src[pairq][ci][:].bitcast(F32)
                        nc.vector.tensor_add(y_dst[pairq][ci][:],
                                             b_t[:, cs], xp)
                        if lq == NL - 1:
                            # stream each finished block out immediately
                            nc.sync.dma_start(out=y_d[pairq, ci],
                                              in_=y_dst[pairq][ci][:])

            for l in range(NL):
                x_cur, y_cur = (xA, xB) if l % 2 == 0 else (xB, xA)
                for d in range(2):
                    for j in range(NPAIR):
                        pair = d * NPAIR + j

                        def mm(ps, g):
                            wt = wp.tile([128, KT * 128], F32R, tag="w")
                            nc.sync.dma_start(out=wt[:], in_=w_d[l, pair, g])
                            for c in range(NCH):
                                cs = slice(c * MMN, (c + 1) * MMN)
                                for k in range(KT):
                                    nc.tensor.matmul(
                                        ps[:, cs],
                                        wt[:, k * 128:(k + 1) * 128],
                                        x_cur[k][c][:],
                                        start=(k == 0), stop=(k == KT - 1))

                        ps_f = pp.tile([128, T], F32, tag="ps")
                        mm(ps_f, 1)
                        ps_xt = pp.tile([128, T], F32, tag="ps")
                        mm(ps_xt, 0)

                        f_t = ew.tile([128, T], F32, tag="f")
                        b_t = ew.tile([128, T], F32, tag="b")
                        r_t = ew.tile([128, T], F32, tag="r")

                        colf = (l * 2 * NPAIR + pair) * 2
                        colr = colf + 1
                        # f = sigmoid(u_f + bf); b = (1-f)*xt = sigmoid(-u_f-bf)*xt
                        nc.scalar.activation(f_t[:], ps_f[:], AF.Sigmoid,
                                             bias=barr[:, colf:colf + 1])
                        nc.scalar.activation(b_t[:], ps_f[:], AF.Sigmoid,
                                             bias=barrn[:, colf:colf + 1],
                                             scale=-1.0)
                        nc.vector.tensor_mul(b_t[:], b_t[:], ps_xt[:])

                        ps_r = pp.tile([128, T], F32, tag="ps")
                        mm(ps_r, 2)

                        if pend[0] is not None:
                            tail(pend[0])

                        nc.scalar.activation(r_t[:], ps_r[:], AF.Sigmoid,
                                             bias=barr[:, colr:colr + 1])

                        pend[0] = (f_t, b_t, r_t, d, l, pair)
                # flush before the next layer reads this layer's outputs
                tail(pend[0])
                pend[0] = None

    nc.finalize()
    return nc


def _get_nc():
    global _cached
    if _cached is None:
        _cached = _build()
    return _cached


def _prep_inputs(rnn_input, emb, Ws, bs):
    """Host-side input arrangement -> per-core in_maps."""
    rnn_input = np.asarray(rnn_input)
    emb = np.asarray(emb, dtype=np.float32)
    Ws = np.asarray(Ws, dtype=np.float32)
    bs = np.asarray(bs, dtype=np.float32)

    x_full = emb[rnn_input]  # (L, B, D)

    # W: (NL, D, 6n) -> (l, pair, g, p, k*128) with col = d*3n + g*n + j*128 + c
    w_arr = np.empty((NL, 2 * NPAIR, 3, 128, KT * 128), np.float32)
    wv = Ws.reshape(NL, KT, 128, 6 * NOUT)  # (l, k, p, col)
    for d in range(2):
        for j in range(NPAIR):
            pair = d * NPAIR + j
            for g in range(3):
                base = d * 3 * NOUT + g * NOUT + j * 128
                blk = wv[:, :, :, base:base + 128]  # (NL, KT, 128p, 128c)
                # dest free index = k*128 + c, partition = p
                w_arr[:, pair, g] = blk.transpose(0, 2, 1, 3).reshape(
                    NL, 128, KT * 128)

    # biases: bs (NL, 4n): bf = [:2n], br = [2n:]
    b_arr = np.empty((128, NL * 2 * NPAIR * 2), np.float32)
    for l in range(NL):
        for d in range(2):
            for j in range(NPAIR):
                pair = d * NPAIR + j
                col = (l * 2 * NPAIR + pair) * 2
                b_arr[:, col] = bs[l, d * NOUT + j * 128:d * NOUT + (j + 1) * 128]
                b_arr[:, col + 1] = bs[l, 2 * NOUT + d * NOUT + j * 128:
                                       2 * NOUT + d * NOUT + (j + 1) * 128]

    in_maps = []
    for i in range(NCORES):
        xc = x_full[:, i * BL:(i + 1) * BL, :]       # (L, BL, D)
        xc = xc.transpose(2, 1, 0).reshape(KT, 128, NCH, MMN)
        xc = xc.transpose(0, 2, 1, 3)                # (k, c, p, t)
        in_maps.append({"x0": np.ascontiguousarray(xc), "w": w_arr, "bias": b_arr})
    return in_maps


def _assemble(results):
    xs, hs = [], []
    for res in results:
        y = res["y"].astype(np.float32)      # (KT, NCH, 128, MMN)
        y = y.transpose(0, 2, 1, 3).reshape(KT, 128, T)
        h = res["hid"].astype(np.float32)    # (NL, pair, 128, BL)
        xs.append(y.reshape(KT, 128, BL, L).transpose(3, 2, 0, 1).reshape(L, BL, D))
        hs.append(h.transpose(0, 3, 1, 2).reshape(NL, BL, D))
    x_out = np.concatenate(xs, axis=1)   # (L, B, D)
    hid_out = np.concatenate(hs, axis=1)  # (NL, B, D)
    return x_out, hid_out


def kernel(rnn_input, input_lengths, emb, Ws, bs):
    from concourse.bass_utils import run_bass_kernel_spmd

    in_maps = _prep_inputs(rnn_input, emb, Ws, bs)
    nc = _get_nc()
    out = run_bass_kernel_spmd(nc, in_maps, list(range(NCORES)))
    return _assemble(out.results)


# revision 35
# speedup vs baseline: 1.0788x; 1.0049x over previous
"""Bidirectional 4-layer SRU encoder on 8 TRN2 NeuronCores.

Strategy: data-parallel over batch (B=16 -> 2 per core). Per core and layer:
  u = x @ W computed on the tensor engine in fp32r (1 cycle/row, ~1.5e-4 rel
  err), with hidden channels on PSUM partitions and tokens (b, t) on the free
  dim. The SRU cell recurrence c[t] = f[t]*c[t-1] + (1-f[t])*xt[t] maps onto
  the DVE tensor_tensor_scan instruction along the free dim; the backward
  direction runs the same scan through reversed (negative-stride) APs.
  Activations (sigmoid/tanh) on the scalar engine, remaining elementwise on
  DVE/GpSimd.

Pipelining: per (direction, unit-group) pair the matmul order is u_f, u_xt,
  u_r so each PSUM buffer's consumers finish before the tensor engine needs
  the slot again, and the post-scan elementwise tail (tanh/sub/mul/add) of
  pair p is emitted during pair p+1 so it never head-of-line blocks the
  scalar engine's sigmoids.

Layouts (per core):
  x buffer  : [128 part, KT=8, T=2048] fp32r, channel c = k*128+p, free = b*L+t
  W         : [l, pair, g, p, k*128] fp32r; pair=(dir*4+j) selects 128 output
              units, g in {xt,f,r}; one contiguous 512KB chunk per quantity
  biases    : [128, (l*8+pair)*2 + {f,r}] fp32
Outputs: y [KT,128,T] fp32r bits (== fp32), hid [NL, pair, 128, BL] fp32.
Embedding gather happens host-side (pure memory reshuffle).
"""
import sys

sys.path.insert(0, "/opt/trn_rl_repo")

import numpy as np

L = 1024          # sequence length
B = 16            # global batch
NCORES = 8
BL = B // NCORES  # batch per core = 2
NOUT = 512        # hidden per direction
D = 2 * NOUT      # input dim = 1024
NL = 4            # layers
KT = D // 128     # contraction k-tiles = 8
NPAIR = NOUT // 128  # 128-unit groups per direction = 4
T = BL * L        # tokens per core = 2048
MMN = 512         # matmul moving free size
NCH = (T + MMN - 1) // MMN  # free chunks = 4

_cached = None


def _rev(ap_mod, ap):
    """Reversed view along the (single) free dim of a 2D [P, F] AP."""
    (pstep, pcount), (fstep, fcount) = ap.ap
    return ap_mod.AP(
        tensor=ap.tensor,
        offset=ap.offset + fstep * (fcount - 1),
        ap=[[pstep, pcount], [-fstep, fcount]],
    )


def _build():
    import concourse.bass as bass
    import concourse.bacc as bacc
    import concourse.mybir as mybir
    from concourse.tile import TileContext

    F32 = mybir.dt.float32
    F32R = mybir.dt.float32r
    AF = mybir.ActivationFunctionType
    OP = mybir.AluOpType

    nc = bacc.Bacc()
    x0_d = nc.declare_dram_parameter("x0", [KT, NCH, 128, MMN], F32R,
                                     isOutput=False)
    w_d = nc.declare_dram_parameter("w", [NL, 2 * NPAIR, 3, 128, KT * 128], F32R,
                                    isOutput=False)
    b_d = nc.declare_dram_parameter("bias", [128, NL * 2 * NPAIR * 2], F32,
                                    isOutput=False)
    y_d = nc.declare_dram_parameter("y", [KT, NCH, 128, MMN], F32R,
                                    isOutput=True)
    h_d = nc.declare_dram_parameter("hid", [NL, 2 * NPAIR, 128, BL], F32,
                                    isOutput=True)

    with TileContext(nc) as tc:
        with (
            tc.tile_pool(name="persist", bufs=1) as persist,
            tc.tile_pool(name="wp", bufs=3) as wp,
            tc.tile_pool(name="pp", bufs=2, space="PSUM") as pp,
            tc.tile_pool(name="ew", bufs=2) as ew,
        ):
            # per-(k, chunk) x buffers: Tile tracks SBUF deps at tile
            # granularity, so separate small tiles let each matmul start as
            # soon as the specific 256KB block it reads is resident
            # (prologue ramp + layer boundaries)
            xA = [[persist.tile([128, MMN], F32R, tag=f"xA{k}_{c}",
                                name=f"xA{k}_{c}") for c in range(NCH)]
                  for k in range(KT)]
            xB = [[persist.tile([128, MMN], F32R, tag=f"xB{k}_{c}",
                                name=f"xB{k}_{c}") for c in range(NCH)]
                  for k in range(KT)]
            barr = persist.tile([128, NL * 2 * NPAIR * 2], F32, tag="barr")
            barrn = persist.tile([128, NL * 2 * NPAIR * 2], F32, tag="barrn")

            # chunk-0 blocks first so the first matmul chains start after
            # ~2MB; issue spread across the three DMA-capable engine queues
            dma_eng = [nc.sync, nc.scalar, nc.gpsimd]
            for c in range(NCH):
                for k in range(KT):
                    dma_eng[k % 3].dma_start(out=xA[k][c][:], in_=x0_d[k, c])
            nc.scalar.dma_start(out=barr[:], in_=b_d[:])
            nc.vector.tensor_scalar_mul(barrn[:], barr[:], -1.0)

            # one-pair-delayed stage: scans + elementwise tail of pair p run
            # during pair p+1 so the only PSUM-gating consumers are the
            # shallow sigmoid/bscan reads right after each matmul block
            pend = [None]

            def tail(st, flush=False):
                f_t, b_t, r_t, d, lq, pairq = st
                c_t = ew.tile([128, T], F32, tag="c")
                x_src, y_dst = (xA, xB) if lq % 2 == 0 else (xB, xA)
                off = (L - 1) if d == 0 else 0
                hsrc = c_t[:].rearrange("p (b t) -> p b t", t=L)

                def seg_scan(b):
                    seg = slice(b * L, (b + 1) * L)
                    o_ap, f_ap, b_ap = c_t[:, seg], f_t[:, seg], b_t[:, seg]
                    if d == 1:
                        o_ap, f_ap, b_ap = (_rev(bass, o_ap), _rev(bass, f_ap),
                                            _rev(bass, b_ap))
                    # c[t] = f[t]*c[t-1] + b[t]  (reversed for d=1)
                    nc.vector.tensor_tensor_scan(
                        o_ap, f_ap, b_ap, 0.0, OP.mult, OP.add)

                def seg_tail(b):
                    # h = r*tanh(c) + (1-r)*xp = r*(tanh(c)-xp) + xp
                    # tanh lands in the dead b_t buffer: an in-place write to
                    # c_t would WAR-block the scalar engine behind the hid DMA
                    seg = slice(b * L, (b + 1) * L)
                    nc.scalar.activation(b_t[:, seg], c_t[:, seg], AF.Tanh)
                    for ci in (2 * b, 2 * b + 1):
                        cs = slice(ci * MMN, (ci + 1) * MMN)
                        xp = x_src[pairq][ci][:].bitcast(F32)
                        # flush path keeps the whole chain on DVE: a GpSimd
                        # hop costs ~2x and sits on the layer-boundary path
                        eng = nc.vector if flush else nc.gpsimd
                        eng.tensor_sub(b_t[:, cs], b_t[:, cs], xp)
                    nc.vector.tensor_mul(b_t[:, seg], r_t[:, seg], b_t[:, seg])
                    for ci in (2 * b, 2 * b + 1):
                        cs = slice(ci * MMN, (ci + 1) * MMN)
                        xp = x_src[pairq][ci][:].bitcast(F32)
                        nc.vector.tensor_add(y_dst[pairq][ci][:],
                                             b_t[:, cs], xp)
                        if lq == NL - 1:
                            # stream each finished block out immediately
                            nc.sync.dma_start(out=y_d[pairq, ci],
                                              in_=y_dst[pairq][ci][:])

                if flush:
                    # interleave per segment so the first half of y lands
                    # (and unblocks the next layer's first chunks) early
                    seg_scan(0)
                    seg_tail(0)
                    seg_scan(1)
                    nc.sync.dma_start(out=h_d[lq, pairq],
                                      in_=hsrc[:, :, off:off + 1].squeeze(-1))
                    seg_tail(1)
                else:
                    for b in range(BL):
                        seg_scan(b)
                    nc.sync.dma_start(out=h_d[lq, pairq],
                                      in_=hsrc[:, :, off:off + 1].squeeze(-1))
                    for b in range(BL):
                        seg_tail(b)

            for l in range(NL):
                x_cur, y_cur = (xA, xB) if l % 2 == 0 else (xB, xA)
                for d in range(2):
                    for j in range(NPAIR):
                        pair = d * NPAIR + j

                        def mm(ps, g):
                            wt = wp.tile([128, KT * 128], F32R, tag="w")
                            nc.sync.dma_start(out=wt[:], in_=w_d[l, pair, g])
                            for c in range(NCH):
                                cs = slice(c * MMN, (c + 1) * MMN)
                                for k in range(KT):
                                    nc.tensor.matmul(
                                        ps[:, cs],
                                        wt[:, k * 128:(k + 1) * 128],
                                        x_cur[k][c][:],
                                        start=(k == 0), stop=(k == KT - 1))

                        ps_f = pp.tile([128, T], F32, tag="ps")
                        mm(ps_f, 1)
                        ps_xt = pp.tile([128, T], F32, tag="ps")
                        mm(ps_xt, 0)

                        f_t = ew.tile([128, T], F32, tag="f")
                        b_t = ew.tile([128, T], F32, tag="b")
                        r_t = ew.tile([128, T], F32, tag="r")

                        colf = (l * 2 * NPAIR + pair) * 2
                        colr = colf + 1
                        # f = sigmoid(u_f + bf); b = (1-f)*xt = sigmoid(-u_f-bf)*xt
                        nc.scalar.activation(f_t[:], ps_f[:], AF.Sigmoid,
                                             bias=barr[:, colf:colf + 1])
                        nc.scalar.activation(b_t[:], ps_f[:], AF.Sigmoid,
                                             bias=barrn[:, colf:colf + 1],
                                             scale=-1.0)
                        nc.vector.tensor_mul(b_t[:], b_t[:], ps_xt[:])

                        ps_r = pp.tile([128, T], F32, tag="ps")
                        mm(ps_r, 2)

                        if pend[0] is not None:
                            tail(pend[0])

                        if pair == 2 * NPAIR - 1:
                            # last pair of the layer: per-chunk sigmoids so r
                            # is ready as each PSUM bank's k-chain completes,
                            # shortening the layer-boundary flush chain
                            for c in range(NCH):
                                cs = slice(c * MMN, (c + 1) * MMN)
                                nc.scalar.activation(
                                    r_t[:, cs], ps_r[:, cs], AF.Sigmoid,
                                    bias=barr[:, colr:colr + 1])
                        else:
                            nc.scalar.activation(r_t[:], ps_r[:], AF.Sigmoid,
                                                 bias=barr[:, colr:colr + 1])

                        pend[0] = (f_t, b_t, r_t, d, l, pair)
                # flush before the next layer reads this layer's outputs
                tail(pend[0], flush=True)
                pend[0] = None

    nc.finalize()
    return nc


def _get_nc():
    global _cached
    if _cached is None:
        _cached = _build()
    return _cached


def _prep_inputs(rnn_input, emb, Ws, bs):
    """Host-side input arrangement -> per-core in_maps."""
    rnn_input = np.asarray(rnn_input)
    emb = np.asarray(emb, dtype=np.float32)
    Ws = np.asarray(Ws, dtype=np.float32)
    bs = np.asarray(bs, dtype=np.float32)

    x_full = emb[rnn_input]  # (L, B, D)

    # W: (NL, D, 6n) -> (l, pair, g, p, k*128) with col = d*3n + g*n + j*128 + c
    w_arr = np.empty((NL, 2 * NPAIR, 3, 128, KT * 128), np.float32)
    wv = Ws.reshape(NL, KT, 128, 6 * NOUT)  # (l, k, p, col)
    for d in range(2):
        for j in range(NPAIR):
            pair = d * NPAIR + j
            for g in range(3):
                base = d * 3 * NOUT + g * NOUT + j * 128
                blk = wv[:, :, :, base:base + 128]  # (NL, KT, 128p, 128c)
                # dest free index = k*128 + c, partition = p
                w_arr[:, pair, g] = blk.transpose(0, 2, 1, 3).reshape(
                    NL, 128, KT * 128)

    # biases: bs (NL, 4n): bf = [:2n], br = [2n:]
    b_arr = np.empty((128, NL * 2 * NPAIR * 2), np.float32)
    for l in range(NL):
        for d in range(2):
            for j in range(NPAIR):
                pair = d * NPAIR + j
                col = (l * 2 * NPAIR + pair) * 2
                b_arr[:, col] = bs[l, d * NOUT + j * 128:d * NOUT + (j + 1) * 128]
                b_arr[:, col + 1] = bs[l, 2 * NOUT + d * NOUT + j * 128:
                                       2 * NOUT + d * NOUT + (j + 1) * 128]

    in_maps = []
    for i in range(NCORES):
        xc = x_full[:, i * BL:(i + 1) * BL, :]       # (L, BL, D)
        xc = xc.transpose(2, 1, 0).reshape(KT, 128, NCH, MMN)
        xc = xc.transpose(0, 2, 1, 3)                # (k, c, p, t)
        in_maps.append({"x0": np.ascontiguousarray(xc), "w": w_arr, "bias": b_arr})
    return in_maps


def _assemble(results):
    xs, hs = [], []
    for res in results:
        y = res["y"].astype(np.float32)      # (KT, NCH, 128, MMN)
        y = y.transpose(0, 2, 1, 3).reshape(KT, 128, T)
        h = res["hid"].astype(np.float32)    # (NL, pair, 128, BL)
        xs.append(y.reshape(KT, 128, BL, L).transpose(3, 2, 0, 1).reshape(L, BL, D))
        hs.append(h.transpose(0, 3, 1, 2).reshape(NL, BL, D))
    x_out = np.concatenate(xs, axis=1)   # (L, B, D)
    hid_out = np.concatenate(hs, axis=1)  # (NL, B, D)
    return x_out, hid_out


def kernel(rnn_input, input_lengths, emb, Ws, bs):
    from concourse.bass_utils import run_bass_kernel_spmd

    in_maps = _prep_inputs(rnn_input, emb, Ws, bs)
    nc = _get_nc()
    out = run_bass_kernel_spmd(nc, in_maps, list(range(NCORES)))
    return _assemble(out.results)
